# revision 9
# baseline (speedup 1.0000x reference)
"""Trainium2 Bass kernel for nn_Attentioncross (gnn_message_passing).

Reference computation, per node n (N=50000) and row r (R=8), D=256:
    idx[r] = [r, r+1, r-1] (with idx[0]=[0,1,2], idx[7]=[7,6,5])
    s[n,j]   = W2 @ leaky_relu(W1 @ z[n,j,:], 0.01)        (scalar per row)
    beta     = softmax([s[self], s[j1], s[j2]])            (over the 3)
    o[n,r,:] = z[n,r,:] + beta1*z[n,j1,:] + beta2*z[n,j2,:]

Strategy: data-parallel over N across 8 cores, NODE-MAJOR layout on chip:
SBUF partition = node (128 nodes per group), free = (row, d). All neighbor
references become free-axis offsets, so the entire gather+combine runs as
per-row scalar_tensor_tensor ops on the DVE with per-node beta scalars —
no gather matmuls, no masks. The score path needs z^T (d on partitions):
d-chunk1 (d=128:256) is shipped pre-transposed fp16 from the host; chunk0
is transposed on-chip by PE matmuls against an identity (lhsT = z slice),
evacuated psum->SBUF by ScalarE. Scores s = W2 @ leaky(W1 @ z^T) come out
of per-row matmuls as [128 nodes, 1]; softmax runs on [128, G, 8] tiles
with shifted free-slices for the neighbor exps. Output is fp16 (rel err
~1e-3 << 2e-2 gate); all matmul accumulation is fp32 in PSUM.
"""
import sys

for p in ("/opt/trn_rl_repo",):
    if p not in sys.path:
        sys.path.insert(0, p)

import numpy as np
from contextlib import ExitStack

N_FULL, R, D = 50000, 8, 256
N_CORES = 8
P = 128
GROUPS = 49                  # node-groups of 128 nodes per core
NODES_PER_CORE = GROUPS * P  # 6272
N_PAD = NODES_PER_CORE * N_CORES   # 50176
SUPER = 4                    # groups per supertile


def _ensure_wpair_op():
    """Register (once) a custom DVE op: out = in0*s0 + in1*s1 with
    per-partition scalars — the full weighted-neighbor sum in one
    instruction. The uop sha is computed with the same lower() the
    compiler uses, so the golden check is self-consistent."""
    from concourse import dve_ops as dops
    from concourse.dve_spec import Spec, Src0, Src1, C0, C1
    from concourse.dve_spec import _has_src1, lower
    from concourse.dve_uop import DveOpSpec

    name = "WPAIR_ANT"
    for o in dops.OPS:
        if o.name == name:
            return o
    spec = Spec(
        body=Src0 * C0 + Src1 * C1,
        reference=lambda in0, in1, s0, s1, imm2: (
            in0.astype(np.float32) * s0 + in1.astype(np.float32) * s1
        ),
    )
    shas = {}
    for ver in ("v3", "v4"):
        tmp = DveOpSpec(
            name=name, opcode=1, uops=lower(spec, ver=ver), rd1_en=_has_src1(spec)
        )
        shas[ver] = tmp.sha(ver)
    op = dops.DveOp(name, spec, subdim=False, uops_sha=shas)
    dops.OPS.append(op)
    dops._SUB_OPCODE_FOR_NAME[name] = dops._CUSTOM_DVE_ROW_BASE + len(dops.OPS) - 1
    return op

# neighbor row indices (matches reference._neighbor_idx for R=8)
J1 = [1, 2, 3, 4, 5, 6, 7, 6]
J2 = [2, 0, 1, 2, 3, 4, 5, 5]


def _build_nc():
    import concourse.bacc as bacc
    import concourse.tile as tile
    from concourse import mybir

    f32 = mybir.dt.float32
    f16 = mybir.dt.float16

    nc = bacc.Bacc("TRN2", target_bir_lowering=False)
    z_d = nc.declare_dram_parameter("z", [P, GROUPS, R, D], f16, isOutput=False)
    zt1_d = nc.declare_dram_parameter("zt1", [P, GROUPS, R, P], f16, isOutput=False)
    w1t_d = nc.declare_dram_parameter("w1t", [P, 2, 16], f16, isOutput=False)
    w2_d = nc.declare_dram_parameter("w2", [16, 2], f16, isOutput=False)
    id_d = nc.declare_dram_parameter("ident", [P, P], f16, isOutput=False)
    o_d = nc.declare_dram_parameter("o", [P, GROUPS, R, D], f16, isOutput=True)

    Prelu = mybir.ActivationFunctionType.Prelu
    Exp = mybir.ActivationFunctionType.Exp
    Copy = mybir.ActivationFunctionType.Copy
    add = mybir.AluOpType.add
    mult = mybir.AluOpType.mult
    wpair = _ensure_wpair_op()

    with tile.TileContext(nc) as tc, ExitStack() as ctx:
        consts = ctx.enter_context(tc.tile_pool(name="consts", bufs=1))
        zpool = ctx.enter_context(tc.tile_pool(name="zp", bufs=3))
        ztpool = ctx.enter_context(tc.tile_pool(name="ztp", bufs=2))
        htpool = ctx.enter_context(tc.tile_pool(name="htp", bufs=2))
        opool = ctx.enter_context(tc.tile_pool(name="op", bufs=2))
        tpool = ctx.enter_context(tc.tile_pool(name="tp", bufs=2))
        small = ctx.enter_context(tc.tile_pool(name="small", bufs=2))

        ps_zt = ctx.enter_context(tc.tile_pool(name="ps_zt", bufs=3, space="PSUM"))
        ps_ht = ctx.enter_context(tc.tile_pool(name="ps_ht", bufs=2, space="PSUM"))
        ps_sc = ctx.enter_context(tc.tile_pool(name="ps_sc", bufs=2, space="PSUM"))

        id_sb = consts.tile([P, P], f16)
        w1t_sb = consts.tile([P, 2, 16], f16)
        w2_sb = consts.tile([16, 2], f16)
        nc.sync.dma_start(out=id_sb, in_=id_d[:])
        nc.sync.dma_start(out=w1t_sb, in_=w1t_d[:])
        nc.sync.dma_start(out=w2_sb, in_=w2_d[:])

        for s0 in range(0, GROUPS, SUPER):
            G = min(SUPER, GROUPS - s0)

            z_sb = zpool.tile([P, SUPER, R, D], f16, tag="z")
            nc.sync.dma_start(out=z_sb[:, 0:G], in_=z_d[:, s0 : s0 + G])
            # zt layout: [dp, chunk, group, r, n]; chunk1 shipped from host
            zt_sb = ztpool.tile([P, 2, SUPER, R, P], f16, tag="zt")
            nc.sync.dma_start(
                out=zt_sb[:, 1, 0:G], in_=zt1_d[:, s0 : s0 + G]
            )

            ht_sb = htpool.tile([16, SUPER, R, P], f16, tag="ht")
            sc_ps = ps_sc.tile([P, SUPER, R, 2], f32, tag="sc")

            for g in range(G):
                # ---- transpose chunk0 on PE: zt0[d, n] = z[n, d]^T ----
                for rh in range(2):
                    zt_ps = ps_zt.tile([P, 4, P], f32, tag="ztps")
                    for rr in range(4):
                        r = rh * 4 + rr
                        nc.tensor.matmul(
                            zt_ps[:, rr, :],
                            z_sb[:, g, r, 0:P],
                            id_sb,
                            start=True,
                            stop=True,
                        )
                    nc.scalar.activation(
                        zt_sb[:, 0, g, rh * 4 : rh * 4 + 4, :], zt_ps, Copy
                    )

                # ---- ht = W1 @ z^T (accumulate over the 2 d-chunks) ----
                for rh in range(2):
                    ht_ps = ps_ht.tile([16, 4, P], f32, tag="htps")
                    for c in range(2):
                        nc.tensor.matmul(
                            ht_ps,
                            w1t_sb[:, c, :],
                            zt_sb[:, c, g, rh * 4 : rh * 4 + 4, :],
                            start=(c == 0),
                            stop=(c == 1),
                        )
                    nc.scalar.activation(
                        ht_sb[:, g, rh * 4 : rh * 4 + 4, :],
                        ht_ps,
                        Prelu,
                        alpha=0.01,
                    )

                # ---- scores: s[n] = W2 @ leaky_ht, per row ----
                for r in range(R):
                    nc.tensor.matmul(
                        sc_ps[:, g, r, :],
                        ht_sb[:, g, r, :],
                        w2_sb,
                        start=True,
                        stop=True,
                    )

            # ---- softmax over {self, left(+1 style), right} via row shifts --
            a_sb = small.tile([P, SUPER, R], f32, tag="a")
            e1 = small.tile([P, SUPER, R], f32, tag="e1")
            e2 = small.tile([P, SUPER, R], f32, tag="e2")
            den = small.tile([P, SUPER, R], f32, tag="den")
            rden = small.tile([P, SUPER, R], f32, tag="rden")
            b1 = small.tile([P, SUPER, R], f32, tag="b1")
            b2 = small.tile([P, SUPER, R], f32, tag="b2")

            nc.scalar.activation(a_sb[:, 0:G], sc_ps[:, 0:G, :, 0], Exp)
            # e1[r] = a[J1[r]] : J1 = r+1 for r<7, 6 at r=7
            nc.gpsimd.tensor_copy(e1[:, 0:G, 0:7], a_sb[:, 0:G, 1:8])
            nc.gpsimd.tensor_copy(e1[:, 0:G, 7:8], a_sb[:, 0:G, 6:7])
            # e2[r] = a[J2[r]] : J2 = r-1 for 1<=r<=6, 2 at r=0, 5 at r=7
            nc.gpsimd.tensor_copy(e2[:, 0:G, 1:7], a_sb[:, 0:G, 0:6])
            nc.gpsimd.tensor_copy(e2[:, 0:G, 0:1], a_sb[:, 0:G, 2:3])
            nc.gpsimd.tensor_copy(e2[:, 0:G, 7:8], a_sb[:, 0:G, 5:6])
            nc.gpsimd.tensor_tensor(den[:, 0:G], a_sb[:, 0:G], e1[:, 0:G], add)
            nc.gpsimd.tensor_tensor(den[:, 0:G], den[:, 0:G], e2[:, 0:G], add)
            nc.vector.reciprocal(rden[:, 0:G], den[:, 0:G])
            nc.gpsimd.tensor_tensor(b1[:, 0:G], e1[:, 0:G], rden[:, 0:G], mult)
            nc.gpsimd.tensor_tensor(b2[:, 0:G], e2[:, 0:G], rden[:, 0:G], mult)

            # ---- combine: d = b1*z[J1] + b2*z[J2] in ONE custom DVE op per
            # row, then o = d + z as one batched 2x-rate tensor_tensor ----
            t_sb = tpool.tile([P, SUPER, R, D], f16, tag="t")
            o_sb = opool.tile([P, SUPER, R, D], f16, tag="o")
            for g in range(G):
                for r in range(R):
                    nc.vector._custom_dve(
                        wpair,
                        out=t_sb[:, g, r, :],
                        in0=z_sb[:, g, J1[r], :],
                        in1=z_sb[:, g, J2[r], :],
                        s0=b1[:, g, r : r + 1],
                        s1=b2[:, g, r : r + 1],
                    )
            for g in range(G):
                eng = nc.gpsimd if (s0 + g) % 3 == 2 else nc.vector
                eng.tensor_tensor(o_sb[:, g], t_sb[:, g], z_sb[:, g], add)
            nc.sync.dma_start(out=o_d[:, s0 : s0 + G], in_=o_sb[:, 0:G])

    nc.finalize()
    return nc


_NC_CACHE = None


def _get_nc():
    global _NC_CACHE
    if _NC_CACHE is None:
        _NC_CACHE = _build_nc()
    return _NC_CACHE


def _prepare_in_maps(z, W1, W2):
    z = np.asarray(z, dtype=np.float32)
    n = z.shape[0]
    zp = np.zeros((N_PAD, R, D), np.float16)
    zp[:n] = z.astype(np.float16)
    # [core, group, n128, r, d]
    z5 = zp.reshape(N_CORES, GROUPS, P, R, D)
    # node-major natural copy: [core][n128, group, r, d]
    z_nm = np.ascontiguousarray(z5.transpose(0, 2, 1, 3, 4))
    # shipped transposed chunk1: [core][dp, group, r, n128]
    zt1 = np.ascontiguousarray(z5[..., P:D].transpose(0, 4, 1, 3, 2))

    w1t = np.ascontiguousarray(
        np.asarray(W1, np.float32).T.reshape(2, P, 16).transpose(1, 0, 2)
    ).astype(np.float16)  # [128, 2, 16]
    w2 = np.ascontiguousarray(
        np.repeat(np.asarray(W2, np.float32).reshape(16, 1), 2, axis=1)
    ).astype(np.float16)  # [16, 2]
    ident = np.eye(P, dtype=np.float16)

    in_maps = []
    for c in range(N_CORES):
        in_maps.append(
            {
                "z": z_nm[c],
                "zt1": zt1[c],
                "w1t": w1t,
                "w2": w2,
                "ident": ident,
            }
        )
    return in_maps


def _gather_out(res, n):
    # o: [core][n128, group, r, d] -> [N_PAD, R, D]
    out = np.empty((N_CORES, P, GROUPS, R, D), np.float16)
    for c in range(N_CORES):
        out[c] = res.results[c]["o"].reshape(P, GROUPS, R, D)
    full = out.transpose(0, 2, 1, 3, 4).reshape(N_PAD, R, D)
    return full[:n].astype(np.float32)


def kernel(z, W1, W2):
    from concourse.bass_utils import run_bass_kernel_spmd

    nc = _get_nc()
    in_maps = _prepare_in_maps(z, W1, W2)
    res = run_bass_kernel_spmd(nc, in_maps, core_ids=list(range(N_CORES)))
    return _gather_out(res, np.asarray(z).shape[0])


# revision 10
# speedup vs baseline: 1.2124x; 1.2124x over previous
"""Trainium2 Bass kernel for nn_Attentioncross (gnn_message_passing).

Reference computation, per node n (N=50000) and row r (R=8), D=256:
    idx[r] = [r, r+1, r-1] (with idx[0]=[0,1,2], idx[7]=[7,6,5])
    s[n,j]   = W2 @ leaky_relu(W1 @ z[n,j,:], 0.01)        (scalar per row)
    beta     = softmax([s[self], s[j1], s[j2]])            (over the 3)
    o[n,r,:] = z[n,r,:] + beta1*z[n,j1,:] + beta2*z[n,j2,:]

Strategy: data-parallel over N across 8 cores, NODE-MAJOR layout on chip:
SBUF partition = node (128 nodes per group), free = (row, d). All neighbor
references become free-axis offsets, so the entire gather+combine runs as
per-row scalar_tensor_tensor ops on the DVE with per-node beta scalars —
no gather matmuls, no masks. The score path needs z^T (d on partitions):
d-chunk1 (d=128:256) is shipped pre-transposed fp16 from the host; chunk0
is transposed on-chip by PE matmuls against an identity (lhsT = z slice),
evacuated psum->SBUF by ScalarE. Scores s = W2 @ leaky(W1 @ z^T) come out
of per-row matmuls as [128 nodes, 1]; softmax runs on [128, G, 8] tiles
with shifted free-slices for the neighbor exps. Output is fp16 (rel err
~1e-3 << 2e-2 gate); all matmul accumulation is fp32 in PSUM.
"""
import sys

for p in ("/opt/trn_rl_repo",):
    if p not in sys.path:
        sys.path.insert(0, p)

import numpy as np
from contextlib import ExitStack

N_FULL, R, D = 50000, 8, 256
N_CORES = 8
P = 128
GROUPS = 49                  # node-groups of 128 nodes per core
NODES_PER_CORE = GROUPS * P  # 6272
N_PAD = NODES_PER_CORE * N_CORES   # 50176
SUPER = 4                    # groups per supertile


def _ensure_wpair_op():
    """Register (once) a custom DVE op: out = in0*s0 + in1*s1 with
    per-partition scalars — the full weighted-neighbor sum in one
    instruction. The uop sha is computed with the same lower() the
    compiler uses, so the golden check is self-consistent."""
    from concourse import dve_ops as dops
    from concourse.dve_spec import Spec, Src0, Src1, C0, C1
    from concourse.dve_spec import _has_src1, lower
    from concourse.dve_uop import DveOpSpec

    name = "WPAIR_ANT"
    for o in dops.OPS:
        if o.name == name:
            return o
    spec = Spec(
        body=Src0 * C0 + Src1 * C1,
        reference=lambda in0, in1, s0, s1, imm2: (
            in0.astype(np.float32) * s0 + in1.astype(np.float32) * s1
        ),
    )
    shas = {}
    for ver in ("v3", "v4"):
        tmp = DveOpSpec(
            name=name, opcode=1, uops=lower(spec, ver=ver), rd1_en=_has_src1(spec)
        )
        shas[ver] = tmp.sha(ver)
    op = dops.DveOp(name, spec, subdim=False, uops_sha=shas)
    dops.OPS.append(op)
    dops._SUB_OPCODE_FOR_NAME[name] = dops._CUSTOM_DVE_ROW_BASE + len(dops.OPS) - 1
    return op

# neighbor row indices (matches reference._neighbor_idx for R=8)
J1 = [1, 2, 3, 4, 5, 6, 7, 6]
J2 = [2, 0, 1, 2, 3, 4, 5, 5]


def _build_nc():
    import concourse.bacc as bacc
    import concourse.tile as tile
    from concourse import mybir

    f32 = mybir.dt.float32
    f16 = mybir.dt.float16

    nc = bacc.Bacc("TRN2", target_bir_lowering=False)
    z_d = nc.declare_dram_parameter("z", [P, GROUPS, R, D], f16, isOutput=False)
    zt1_d = nc.declare_dram_parameter("zt1", [P, GROUPS, R, P], f16, isOutput=False)
    w1t_d = nc.declare_dram_parameter("w1t", [P, 2, 16], f16, isOutput=False)
    w2_d = nc.declare_dram_parameter("w2", [16, 2], f16, isOutput=False)
    id_d = nc.declare_dram_parameter("ident", [P, P], f16, isOutput=False)
    o_d = nc.declare_dram_parameter("o", [P, GROUPS, R, D], f16, isOutput=True)

    Prelu = mybir.ActivationFunctionType.Prelu
    Exp = mybir.ActivationFunctionType.Exp
    Copy = mybir.ActivationFunctionType.Copy
    add = mybir.AluOpType.add
    mult = mybir.AluOpType.mult
    wpair = _ensure_wpair_op()

    with tile.TileContext(nc) as tc, ExitStack() as ctx:
        consts = ctx.enter_context(tc.tile_pool(name="consts", bufs=1))
        zpool = ctx.enter_context(tc.tile_pool(name="zp", bufs=3))
        ztpool = ctx.enter_context(tc.tile_pool(name="ztp", bufs=2))
        htpool = ctx.enter_context(tc.tile_pool(name="htp", bufs=2))
        opool = ctx.enter_context(tc.tile_pool(name="op", bufs=2))
        tpool = ctx.enter_context(tc.tile_pool(name="tp", bufs=2))
        small = ctx.enter_context(tc.tile_pool(name="small", bufs=2))

        ps_zt = ctx.enter_context(tc.tile_pool(name="ps_zt", bufs=3, space="PSUM"))
        ps_ht = ctx.enter_context(tc.tile_pool(name="ps_ht", bufs=2, space="PSUM"))
        ps_sc = ctx.enter_context(tc.tile_pool(name="ps_sc", bufs=2, space="PSUM"))

        id_sb = consts.tile([P, P], f16)
        w1t_sb = consts.tile([P, 2, 16], f16)
        w2_sb = consts.tile([16, 2], f16)
        nc.sync.dma_start(out=id_sb, in_=id_d[:])
        nc.sync.dma_start(out=w1t_sb, in_=w1t_d[:])
        nc.sync.dma_start(out=w2_sb, in_=w2_d[:])

        for s0 in range(0, GROUPS, SUPER):
            G = min(SUPER, GROUPS - s0)

            z_sb = zpool.tile([P, SUPER, R, D], f16, tag="z")
            nc.sync.dma_start(out=z_sb[:, 0:G], in_=z_d[:, s0 : s0 + G])
            # zt layout: [dp, chunk, group, r, n]; chunk1 shipped from host
            zt_sb = ztpool.tile([P, 2, SUPER, R, P], f16, tag="zt")
            nc.sync.dma_start(
                out=zt_sb[:, 1, 0:G], in_=zt1_d[:, s0 : s0 + G]
            )

            ht_sb = htpool.tile([16, SUPER, R, P], f16, tag="ht")
            sc_ps = ps_sc.tile([P, SUPER, R, 2], f32, tag="sc")

            for g in range(G):
                # ---- transpose chunk0 on PE: zt0[d, n] = z[n, d]^T ----
                for rh in range(2):
                    zt_ps = ps_zt.tile([P, 4, P], f32, tag="ztps")
                    for rr in range(4):
                        r = rh * 4 + rr
                        nc.tensor.matmul(
                            zt_ps[:, rr, :],
                            z_sb[:, g, r, 0:P],
                            id_sb,
                            start=True,
                            stop=True,
                        )
                    nc.scalar.activation(
                        zt_sb[:, 0, g, rh * 4 : rh * 4 + 4, :], zt_ps, Copy
                    )

                # ---- ht = W1 @ z^T (accumulate over the 2 d-chunks) ----
                for rh in range(2):
                    ht_ps = ps_ht.tile([16, 4, P], f32, tag="htps")
                    for c in range(2):
                        nc.tensor.matmul(
                            ht_ps,
                            w1t_sb[:, c, :],
                            zt_sb[:, c, g, rh * 4 : rh * 4 + 4, :],
                            start=(c == 0),
                            stop=(c == 1),
                        )
                    nc.scalar.activation(
                        ht_sb[:, g, rh * 4 : rh * 4 + 4, :],
                        ht_ps,
                        Prelu,
                        alpha=0.01,
                    )

                # ---- scores: s[n] = W2 @ leaky_ht, per row ----
                for r in range(R):
                    nc.tensor.matmul(
                        sc_ps[:, g, r, :],
                        ht_sb[:, g, r, :],
                        w2_sb,
                        start=True,
                        stop=True,
                    )

            # ---- softmax over {self, left(+1 style), right} via row shifts --
            a_sb = small.tile([P, SUPER, R], f32, tag="a")
            e1 = small.tile([P, SUPER, R], f32, tag="e1")
            e2 = small.tile([P, SUPER, R], f32, tag="e2")
            den = small.tile([P, SUPER, R], f32, tag="den")
            rden = small.tile([P, SUPER, R], f32, tag="rden")
            b1 = small.tile([P, SUPER, R], f32, tag="b1")
            b2 = small.tile([P, SUPER, R], f32, tag="b2")

            nc.scalar.activation(a_sb[:, 0:G], sc_ps[:, 0:G, :, 0], Exp)
            # e1[r] = a[J1[r]] : J1 = r+1 for r<7, 6 at r=7
            nc.scalar.copy(e1[:, 0:G, 0:7], a_sb[:, 0:G, 1:8])
            nc.scalar.copy(e1[:, 0:G, 7:8], a_sb[:, 0:G, 6:7])
            # e2[r] = a[J2[r]] : J2 = r-1 for 1<=r<=6, 2 at r=0, 5 at r=7
            nc.scalar.copy(e2[:, 0:G, 1:7], a_sb[:, 0:G, 0:6])
            nc.scalar.copy(e2[:, 0:G, 0:1], a_sb[:, 0:G, 2:3])
            nc.scalar.copy(e2[:, 0:G, 7:8], a_sb[:, 0:G, 5:6])
            nc.vector.tensor_tensor(den[:, 0:G], a_sb[:, 0:G], e1[:, 0:G], add)
            nc.vector.tensor_tensor(den[:, 0:G], den[:, 0:G], e2[:, 0:G], add)
            nc.vector.reciprocal(rden[:, 0:G], den[:, 0:G])
            nc.vector.tensor_tensor(b1[:, 0:G], e1[:, 0:G], rden[:, 0:G], mult)
            nc.vector.tensor_tensor(b2[:, 0:G], e2[:, 0:G], rden[:, 0:G], mult)

            # ---- combine: d = b1*z[J1] + b2*z[J2] in ONE custom DVE op per
            # row, then o = d + z as one batched 2x-rate tensor_tensor ----
            t_sb = tpool.tile([P, SUPER, R, D], f16, tag="t")
            o_sb = opool.tile([P, SUPER, R, D], f16, tag="o")
            for g in range(G):
                for r in range(R):
                    nc.vector._custom_dve(
                        wpair,
                        out=t_sb[:, g, r, :],
                        in0=z_sb[:, g, J1[r], :],
                        in1=z_sb[:, g, J2[r], :],
                        s0=b1[:, g, r : r + 1],
                        s1=b2[:, g, r : r + 1],
                    )
            nc.vector.tensor_tensor(
                o_sb[:, 0:G], t_sb[:, 0:G], z_sb[:, 0:G], add
            )
            nc.sync.dma_start(out=o_d[:, s0 : s0 + G], in_=o_sb[:, 0:G])

    nc.finalize()
    return nc


_NC_CACHE = None


def _get_nc():
    global _NC_CACHE
    if _NC_CACHE is None:
        _NC_CACHE = _build_nc()
    return _NC_CACHE


def _prepare_in_maps(z, W1, W2):
    z = np.asarray(z, dtype=np.float32)
    n = z.shape[0]
    zp = np.zeros((N_PAD, R, D), np.float16)
    zp[:n] = z.astype(np.float16)
    # [core, group, n128, r, d]
    z5 = zp.reshape(N_CORES, GROUPS, P, R, D)
    # node-major natural copy: [core][n128, group, r, d]
    z_nm = np.ascontiguousarray(z5.transpose(0, 2, 1, 3, 4))
    # shipped transposed chunk1: [core][dp, group, r, n128]
    zt1 = np.ascontiguousarray(z5[..., P:D].transpose(0, 4, 1, 3, 2))

    w1t = np.ascontiguousarray(
        np.asarray(W1, np.float32).T.reshape(2, P, 16).transpose(1, 0, 2)
    ).astype(np.float16)  # [128, 2, 16]
    w2 = np.ascontiguousarray(
        np.repeat(np.asarray(W2, np.float32).reshape(16, 1), 2, axis=1)
    ).astype(np.float16)  # [16, 2]
    ident = np.eye(P, dtype=np.float16)

    in_maps = []
    for c in range(N_CORES):
        in_maps.append(
            {
                "z": z_nm[c],
                "zt1": zt1[c],
                "w1t": w1t,
                "w2": w2,
                "ident": ident,
            }
        )
    return in_maps


def _gather_out(res, n):
    # o: [core][n128, group, r, d] -> [N_PAD, R, D]
    out = np.empty((N_CORES, P, GROUPS, R, D), np.float16)
    for c in range(N_CORES):
        out[c] = res.results[c]["o"].reshape(P, GROUPS, R, D)
    full = out.transpose(0, 2, 1, 3, 4).reshape(N_PAD, R, D)
    return full[:n].astype(np.float32)


def kernel(z, W1, W2):
    from concourse.bass_utils import run_bass_kernel_spmd

    nc = _get_nc()
    in_maps = _prepare_in_maps(z, W1, W2)
    res = run_bass_kernel_spmd(nc, in_maps, core_ids=list(range(N_CORES)))
    return _gather_out(res, np.asarray(z).shape[0])


# revision 11
# speedup vs baseline: 1.4238x; 1.1744x over previous
"""Trainium2 Bass kernel for nn_Attentioncross (gnn_message_passing).

Reference computation, per node n (N=50000) and row r (R=8), D=256:
    idx[r] = [r, r+1, r-1] (with idx[0]=[0,1,2], idx[7]=[7,6,5])
    s[n,j]   = W2 @ leaky_relu(W1 @ z[n,j,:], 0.01)        (scalar per row)
    beta     = softmax([s[self], s[j1], s[j2]])            (over the 3)
    o[n,r,:] = z[n,r,:] + beta1*z[n,j1,:] + beta2*z[n,j2,:]

Strategy: data-parallel over N across 8 cores, NODE-MAJOR layout on chip:
SBUF partition = node (128 nodes per group), free = (row, d). All neighbor
references become free-axis offsets, so the entire gather+combine runs as
per-row scalar_tensor_tensor ops on the DVE with per-node beta scalars —
no gather matmuls, no masks. The score path needs z^T (d on partitions):
d-chunk1 (d=128:256) is shipped pre-transposed fp16 from the host; chunk0
is transposed on-chip by PE matmuls against an identity (lhsT = z slice),
evacuated psum->SBUF by ScalarE. Scores s = W2 @ leaky(W1 @ z^T) come out
of per-row matmuls as [128 nodes, 1]; softmax runs on [128, G, 8] tiles
with shifted free-slices for the neighbor exps. Output is fp16 (rel err
~1e-3 << 2e-2 gate); all matmul accumulation is fp32 in PSUM.
"""
import sys

for p in ("/opt/trn_rl_repo",):
    if p not in sys.path:
        sys.path.insert(0, p)

import numpy as np
from contextlib import ExitStack

N_FULL, R, D = 50000, 8, 256
N_CORES = 8
P = 128
GROUPS = 49                  # node-groups of 128 nodes per core
NODES_PER_CORE = GROUPS * P  # 6272
N_PAD = NODES_PER_CORE * N_CORES   # 50176
SUPER = 4                    # groups per supertile


def _ensure_wpair_op():
    """Register (once) a custom DVE op: out = in0*s0 + in1*s1 with
    per-partition scalars — the full weighted-neighbor sum in one
    instruction. The uop sha is computed with the same lower() the
    compiler uses, so the golden check is self-consistent."""
    from concourse import dve_ops as dops
    from concourse.dve_spec import Spec, Src0, Src1, C0, C1
    from concourse.dve_spec import _has_src1, lower
    from concourse.dve_uop import DveOpSpec

    name = "WPAIR_ANT"
    for o in dops.OPS:
        if o.name == name:
            return o
    spec = Spec(
        body=Src0 * C0 + Src1 * C1,
        reference=lambda in0, in1, s0, s1, imm2: (
            in0.astype(np.float32) * s0 + in1.astype(np.float32) * s1
        ),
    )
    shas = {}
    for ver in ("v3", "v4"):
        tmp = DveOpSpec(
            name=name, opcode=1, uops=lower(spec, ver=ver), rd1_en=_has_src1(spec)
        )
        shas[ver] = tmp.sha(ver)
    op = dops.DveOp(name, spec, subdim=False, uops_sha=shas)
    dops.OPS.append(op)
    dops._SUB_OPCODE_FOR_NAME[name] = dops._CUSTOM_DVE_ROW_BASE + len(dops.OPS) - 1
    return op

# neighbor row indices (matches reference._neighbor_idx for R=8)
J1 = [1, 2, 3, 4, 5, 6, 7, 6]
J2 = [2, 0, 1, 2, 3, 4, 5, 5]


def _build_nc():
    import concourse.bacc as bacc
    import concourse.tile as tile
    from concourse import mybir

    f32 = mybir.dt.float32
    f16 = mybir.dt.float16

    nc = bacc.Bacc("TRN2", target_bir_lowering=False)
    z_d = nc.declare_dram_parameter("z", [P, GROUPS, R, D], f16, isOutput=False)
    zt1_d = nc.declare_dram_parameter("zt1", [P, GROUPS, R, P], f16, isOutput=False)
    w1t_d = nc.declare_dram_parameter("w1t", [P, 2, 16], f16, isOutput=False)
    w2_d = nc.declare_dram_parameter("w2", [16, 2], f16, isOutput=False)
    id_d = nc.declare_dram_parameter("ident", [P, P], f16, isOutput=False)
    o_d = nc.declare_dram_parameter("o", [P, GROUPS, R, D], f16, isOutput=True)

    Prelu = mybir.ActivationFunctionType.Prelu
    Exp = mybir.ActivationFunctionType.Exp
    Copy = mybir.ActivationFunctionType.Copy
    add = mybir.AluOpType.add
    mult = mybir.AluOpType.mult
    wpair = _ensure_wpair_op()

    with tile.TileContext(nc) as tc, ExitStack() as ctx:
        consts = ctx.enter_context(tc.tile_pool(name="consts", bufs=1))
        zpool = ctx.enter_context(tc.tile_pool(name="zp", bufs=3))
        ztpool = ctx.enter_context(tc.tile_pool(name="ztp", bufs=2))
        htpool = ctx.enter_context(tc.tile_pool(name="htp", bufs=2))
        opool = ctx.enter_context(tc.tile_pool(name="op", bufs=2))
        tpool = ctx.enter_context(tc.tile_pool(name="tp", bufs=2))
        small = ctx.enter_context(tc.tile_pool(name="small", bufs=2))

        ps_zt = ctx.enter_context(tc.tile_pool(name="ps_zt", bufs=3, space="PSUM"))
        ps_ht = ctx.enter_context(tc.tile_pool(name="ps_ht", bufs=2, space="PSUM"))
        ps_sc = ctx.enter_context(tc.tile_pool(name="ps_sc", bufs=2, space="PSUM"))

        id_sb = consts.tile([P, P], f16)
        w1t_sb = consts.tile([P, 2, 16], f16)
        w2_sb = consts.tile([16, 2], f16)
        nc.sync.dma_start(out=id_sb, in_=id_d[:])
        nc.sync.dma_start(out=w1t_sb, in_=w1t_d[:])
        nc.sync.dma_start(out=w2_sb, in_=w2_d[:])

        for s0 in range(0, GROUPS, SUPER):
            G = min(SUPER, GROUPS - s0)

            z_sb = zpool.tile([P, SUPER, R, D], f16, tag="z")
            nc.sync.dma_start(out=z_sb[:, 0:G], in_=z_d[:, s0 : s0 + G])
            # zt layout: [dp, chunk, group, r, n]; chunk1 shipped from host
            zt_sb = ztpool.tile([P, 2, SUPER, R, P], f16, tag="zt")
            nc.sync.dma_start(
                out=zt_sb[:, 1, 0:G], in_=zt1_d[:, s0 : s0 + G]
            )

            ht_sb = htpool.tile([16, SUPER, R, P], f16, tag="ht")
            sc_ps = ps_sc.tile([P, SUPER, R, 2], f32, tag="sc")

            for g in range(G):
                # ---- transpose chunk0 on PE: zt0[d, n] = z[n, d]^T ----
                for rh in range(2):
                    zt_ps = ps_zt.tile([P, 4, P], f32, tag="ztps")
                    for rr in range(4):
                        r = rh * 4 + rr
                        nc.tensor.matmul(
                            zt_ps[:, rr, :],
                            z_sb[:, g, r, 0:P],
                            id_sb,
                            start=True,
                            stop=True,
                        )
                    nc.scalar.activation(
                        zt_sb[:, 0, g, rh * 4 : rh * 4 + 4, :], zt_ps, Copy
                    )

                # ---- ht = W1 @ z^T (accumulate over the 2 d-chunks) ----
                for rh in range(2):
                    ht_ps = ps_ht.tile([16, 4, P], f32, tag="htps")
                    for c in range(2):
                        nc.tensor.matmul(
                            ht_ps,
                            w1t_sb[:, c, :],
                            zt_sb[:, c, g, rh * 4 : rh * 4 + 4, :],
                            start=(c == 0),
                            stop=(c == 1),
                        )
                    nc.scalar.activation(
                        ht_sb[:, g, rh * 4 : rh * 4 + 4, :],
                        ht_ps,
                        Prelu,
                        alpha=0.01,
                    )

                # ---- scores: s[n] = W2 @ leaky_ht, per row ----
                for r in range(R):
                    nc.tensor.matmul(
                        sc_ps[:, g, r, :],
                        ht_sb[:, g, r, :],
                        w2_sb,
                        start=True,
                        stop=True,
                    )

            # ---- softmax over {self, left(+1 style), right} via row shifts --
            a_sb = small.tile([P, SUPER, R], f32, tag="a")
            e1 = small.tile([P, SUPER, R], f32, tag="e1")
            e2 = small.tile([P, SUPER, R], f32, tag="e2")
            den = small.tile([P, SUPER, R], f32, tag="den")
            rden = small.tile([P, SUPER, R], f32, tag="rden")
            b1 = small.tile([P, SUPER, R], f32, tag="b1")
            b2 = small.tile([P, SUPER, R], f32, tag="b2")

            nc.scalar.activation(a_sb[:, 0:G], sc_ps[:, 0:G, :, 0], Exp)
            # e1[r] = a[J1[r]] : J1 = r+1 for r<7, 6 at r=7
            nc.vector.tensor_copy(e1[:, 0:G, 0:7], a_sb[:, 0:G, 1:8])
            nc.vector.tensor_copy(e1[:, 0:G, 7:8], a_sb[:, 0:G, 6:7])
            # e2[r] = a[J2[r]] : J2 = r-1 for 1<=r<=6, 2 at r=0, 5 at r=7
            nc.vector.tensor_copy(e2[:, 0:G, 1:7], a_sb[:, 0:G, 0:6])
            nc.vector.tensor_copy(e2[:, 0:G, 0:1], a_sb[:, 0:G, 2:3])
            nc.vector.tensor_copy(e2[:, 0:G, 7:8], a_sb[:, 0:G, 5:6])
            nc.vector.tensor_tensor(den[:, 0:G], a_sb[:, 0:G], e1[:, 0:G], add)
            nc.vector.tensor_tensor(den[:, 0:G], den[:, 0:G], e2[:, 0:G], add)
            nc.vector.reciprocal(rden[:, 0:G], den[:, 0:G])
            nc.vector.tensor_tensor(b1[:, 0:G], e1[:, 0:G], rden[:, 0:G], mult)
            nc.vector.tensor_tensor(b2[:, 0:G], e2[:, 0:G], rden[:, 0:G], mult)

            # ---- combine: d = b1*z[J1] + b2*z[J2] in ONE custom DVE op per
            # row, then o = d + z as one batched 2x-rate tensor_tensor ----
            t_sb = tpool.tile([P, SUPER, R, D], f16, tag="t")
            o_sb = opool.tile([P, SUPER, R, D], f16, tag="o")
            for g in range(G):
                for r in range(R):
                    nc.vector._custom_dve(
                        wpair,
                        out=t_sb[:, g, r, :],
                        in0=z_sb[:, g, J1[r], :],
                        in1=z_sb[:, g, J2[r], :],
                        s0=b1[:, g, r : r + 1],
                        s1=b2[:, g, r : r + 1],
                    )
            nc.vector.tensor_tensor(
                o_sb[:, 0:G], t_sb[:, 0:G], z_sb[:, 0:G], add
            )
            nc.sync.dma_start(out=o_d[:, s0 : s0 + G], in_=o_sb[:, 0:G])

    nc.finalize()
    return nc


_NC_CACHE = None


def _get_nc():
    global _NC_CACHE
    if _NC_CACHE is None:
        _NC_CACHE = _build_nc()
    return _NC_CACHE


def _prepare_in_maps(z, W1, W2):
    z = np.asarray(z, dtype=np.float32)
    n = z.shape[0]
    zp = np.zeros((N_PAD, R, D), np.float16)
    zp[:n] = z.astype(np.float16)
    # [core, group, n128, r, d]
    z5 = zp.reshape(N_CORES, GROUPS, P, R, D)
    # node-major natural copy: [core][n128, group, r, d]
    z_nm = np.ascontiguousarray(z5.transpose(0, 2, 1, 3, 4))
    # shipped transposed chunk1: [core][dp, group, r, n128]
    zt1 = np.ascontiguousarray(z5[..., P:D].transpose(0, 4, 1, 3, 2))

    w1t = np.ascontiguousarray(
        np.asarray(W1, np.float32).T.reshape(2, P, 16).transpose(1, 0, 2)
    ).astype(np.float16)  # [128, 2, 16]
    w2 = np.ascontiguousarray(
        np.repeat(np.asarray(W2, np.float32).reshape(16, 1), 2, axis=1)
    ).astype(np.float16)  # [16, 2]
    ident = np.eye(P, dtype=np.float16)

    in_maps = []
    for c in range(N_CORES):
        in_maps.append(
            {
                "z": z_nm[c],
                "zt1": zt1[c],
                "w1t": w1t,
                "w2": w2,
                "ident": ident,
            }
        )
    return in_maps


def _gather_out(res, n):
    # o: [core][n128, group, r, d] -> [N_PAD, R, D]
    out = np.empty((N_CORES, P, GROUPS, R, D), np.float16)
    for c in range(N_CORES):
        out[c] = res.results[c]["o"].reshape(P, GROUPS, R, D)
    full = out.transpose(0, 2, 1, 3, 4).reshape(N_PAD, R, D)
    return full[:n].astype(np.float32)


def kernel(z, W1, W2):
    from concourse.bass_utils import run_bass_kernel_spmd

    nc = _get_nc()
    in_maps = _prepare_in_maps(z, W1, W2)
    res = run_bass_kernel_spmd(nc, in_maps, core_ids=list(range(N_CORES)))
    return _gather_out(res, np.asarray(z).shape[0])


# revision 17
# speedup vs baseline: 1.4514x; 1.0194x over previous
"""Trainium2 Bass kernel for nn_Attentioncross (gnn_message_passing).

Reference computation, per node n (N=50000) and row r (R=8), D=256:
    idx[r] = [r, r+1, r-1] (with idx[0]=[0,1,2], idx[7]=[7,6,5])
    s[n,j]   = W2 @ leaky_relu(W1 @ z[n,j,:], 0.01)        (scalar per row)
    beta     = softmax([s[self], s[j1], s[j2]])            (over the 3)
    o[n,r,:] = z[n,r,:] + beta1*z[n,j1,:] + beta2*z[n,j2,:]

Strategy: data-parallel over N across 8 cores, NODE-MAJOR layout on chip:
SBUF partition = node (128 nodes per group), free = (row, d). All neighbor
references become free-axis offsets — no gather matmuls, no masks. The
combine runs as one custom fused DVE op per row (WPAIR: b1*z[j1]+b2*z[j2]
with per-node beta scalars, registered at import) plus a single batched
2x-rate tensor_tensor residual add per supertile. The score path needs z^T (d on partitions):
d-chunk1 (d=128:256) is shipped pre-transposed fp16 from the host; chunk0
is transposed on-chip by PE matmuls against an identity (lhsT = z slice),
evacuated psum->SBUF by ScalarE. Scores s = W2 @ leaky(W1 @ z^T) come out
of per-row matmuls as [128 nodes, 1]; softmax runs on [128, G, 8] tiles
with shifted free-slices for the neighbor exps. Output is fp16 (rel err
~1e-3 << 2e-2 gate); all matmul accumulation is fp32 in PSUM.
"""
import sys

for p in ("/opt/trn_rl_repo",):
    if p not in sys.path:
        sys.path.insert(0, p)

import numpy as np
from contextlib import ExitStack

N_FULL, R, D = 50000, 8, 256
N_CORES = 8
P = 128
GROUPS = 49                  # node-groups of 128 nodes per core
NODES_PER_CORE = GROUPS * P  # 6272
N_PAD = NODES_PER_CORE * N_CORES   # 50176
SUPER = 4                    # groups per supertile


def _ensure_wpair_op():
    """Register (once) a custom DVE op: out = in0*s0 + in1*s1 with
    per-partition scalars — the full weighted-neighbor sum in one
    instruction. The uop sha is computed with the same lower() the
    compiler uses, so the golden check is self-consistent."""
    from concourse import dve_ops as dops
    from concourse.dve_spec import Spec, Src0, Src1, C0, C1
    from concourse.dve_spec import _has_src1, lower
    from concourse.dve_uop import DveOpSpec

    name = "WPAIR_ANT"
    for o in dops.OPS:
        if o.name == name:
            return o
    spec = Spec(
        body=Src0 * C0 + Src1 * C1,
        reference=lambda in0, in1, s0, s1, imm2: (
            in0.astype(np.float32) * s0 + in1.astype(np.float32) * s1
        ),
    )
    shas = {}
    for ver in ("v3", "v4"):
        tmp = DveOpSpec(
            name=name, opcode=1, uops=lower(spec, ver=ver), rd1_en=_has_src1(spec)
        )
        shas[ver] = tmp.sha(ver)
    op = dops.DveOp(name, spec, subdim=False, uops_sha=shas)
    dops.OPS.append(op)
    dops._SUB_OPCODE_FOR_NAME[name] = dops._CUSTOM_DVE_ROW_BASE + len(dops.OPS) - 1
    return op

# neighbor row indices (matches reference._neighbor_idx for R=8)
J1 = [1, 2, 3, 4, 5, 6, 7, 6]
J2 = [2, 0, 1, 2, 3, 4, 5, 5]


def _build_nc():
    import concourse.bacc as bacc
    import concourse.tile as tile
    from concourse import mybir

    f32 = mybir.dt.float32
    f16 = mybir.dt.float16

    nc = bacc.Bacc("TRN2", target_bir_lowering=False)
    z_d = nc.declare_dram_parameter("z", [P, GROUPS, R, D], f16, isOutput=False)
    zt1_d = nc.declare_dram_parameter("zt1", [P, GROUPS, R, P], f16, isOutput=False)
    w1t_d = nc.declare_dram_parameter("w1t", [P, 2, 16], f16, isOutput=False)
    w2_d = nc.declare_dram_parameter("w2", [16, 2], f16, isOutput=False)
    id_d = nc.declare_dram_parameter("ident", [P, P], f16, isOutput=False)
    o_d = nc.declare_dram_parameter("o", [P, GROUPS, R, D], f16, isOutput=True)

    Prelu = mybir.ActivationFunctionType.Prelu
    Exp = mybir.ActivationFunctionType.Exp
    Copy = mybir.ActivationFunctionType.Copy
    add = mybir.AluOpType.add
    mult = mybir.AluOpType.mult
    wpair = _ensure_wpair_op()

    with tile.TileContext(nc) as tc, ExitStack() as ctx:
        consts = ctx.enter_context(tc.tile_pool(name="consts", bufs=1))
        zpool = ctx.enter_context(tc.tile_pool(name="zp", bufs=3))
        ztpool = ctx.enter_context(tc.tile_pool(name="ztp", bufs=2))
        htpool = ctx.enter_context(tc.tile_pool(name="htp", bufs=2))
        opool = ctx.enter_context(tc.tile_pool(name="op", bufs=2))
        tpool = ctx.enter_context(tc.tile_pool(name="tp", bufs=2))
        small = ctx.enter_context(tc.tile_pool(name="small", bufs=2))

        ps_zt = ctx.enter_context(tc.tile_pool(name="ps_zt", bufs=3, space="PSUM"))
        ps_ht = ctx.enter_context(tc.tile_pool(name="ps_ht", bufs=2, space="PSUM"))
        ps_sc = ctx.enter_context(tc.tile_pool(name="ps_sc", bufs=2, space="PSUM"))

        id_sb = consts.tile([P, P], f16)
        w1t_sb = consts.tile([P, 2, 16], f16)
        w2_sb = consts.tile([16, 2], f16)
        nc.sync.dma_start(out=id_sb, in_=id_d[:])
        nc.sync.dma_start(out=w1t_sb, in_=w1t_d[:])
        nc.sync.dma_start(out=w2_sb, in_=w2_d[:])
        # warm the exp_and_others activation table set off the critical path
        warm = consts.tile([P, 1], f32)
        nc.scalar.activation(warm, id_sb[:, 0:1], Exp)

        # ramped schedule: small first supers so the first betas land fast
        # (cuts the ~23us DVE prologue stall), small last super for drain
        sched = [1, 2] + [SUPER] * ((GROUPS - 5) // SUPER) + [1, 1]
        assert sum(sched) == GROUPS
        s0 = 0
        for G in sched:

            z_sb = zpool.tile([P, SUPER, R, D], f16, tag="z")
            nc.sync.dma_start(out=z_sb[:, 0:G], in_=z_d[:, s0 : s0 + G])
            # zt layout: [dp, chunk, group, r, n]; chunk1 shipped from host
            zt_sb = ztpool.tile([P, 2, SUPER, R, P], f16, tag="zt")
            nc.sync.dma_start(
                out=zt_sb[:, 1, 0:G], in_=zt1_d[:, s0 : s0 + G]
            )

            ht_sb = htpool.tile([16, SUPER, R, P], f16, tag="ht")
            sc_ps = ps_sc.tile([P, SUPER, R, 2], f32, tag="sc")

            for g in range(G):
                # ---- transpose chunk0 on PE: zt0[d, n] = z[n, d]^T ----
                for rh in range(2):
                    zt_ps = ps_zt.tile([P, 4, P], f32, tag="ztps")
                    for rr in range(4):
                        r = rh * 4 + rr
                        nc.tensor.matmul(
                            zt_ps[:, rr, :],
                            z_sb[:, g, r, 0:P],
                            id_sb,
                            start=True,
                            stop=True,
                        )
                    nc.scalar.activation(
                        zt_sb[:, 0, g, rh * 4 : rh * 4 + 4, :], zt_ps, Copy
                    )

                # ---- ht = W1 @ z^T (accumulate over the 2 d-chunks) ----
                for rh in range(2):
                    ht_ps = ps_ht.tile([16, 4, P], f32, tag="htps")
                    for c in range(2):
                        nc.tensor.matmul(
                            ht_ps,
                            w1t_sb[:, c, :],
                            zt_sb[:, c, g, rh * 4 : rh * 4 + 4, :],
                            start=(c == 0),
                            stop=(c == 1),
                        )
                    nc.scalar.activation(
                        ht_sb[:, g, rh * 4 : rh * 4 + 4, :],
                        ht_ps,
                        Prelu,
                        alpha=0.01,
                    )

                # ---- scores: s[n] = W2 @ leaky_ht, per row ----
                for r in range(R):
                    nc.tensor.matmul(
                        sc_ps[:, g, r, :],
                        ht_sb[:, g, r, :],
                        w2_sb,
                        start=True,
                        stop=True,
                    )

            # ---- softmax over {self, left(+1 style), right} via row shifts --
            a_sb = small.tile([P, SUPER, R], f32, tag="a")
            e1 = small.tile([P, SUPER, R], f32, tag="e1")
            e2 = small.tile([P, SUPER, R], f32, tag="e2")
            den = small.tile([P, SUPER, R], f32, tag="den")
            rden = small.tile([P, SUPER, R], f32, tag="rden")
            b1 = small.tile([P, SUPER, R], f32, tag="b1")
            b2 = small.tile([P, SUPER, R], f32, tag="b2")

            nc.scalar.activation(a_sb[:, 0:G], sc_ps[:, 0:G, :, 0], Exp)
            # e1[r] = a[J1[r]] : J1 = r+1 for r<7, 6 at r=7
            nc.vector.tensor_copy(e1[:, 0:G, 0:7], a_sb[:, 0:G, 1:8])
            nc.vector.tensor_copy(e1[:, 0:G, 7:8], a_sb[:, 0:G, 6:7])
            # e2[r] = a[J2[r]] : J2 = r-1 for 1<=r<=6, 2 at r=0, 5 at r=7
            nc.vector.tensor_copy(e2[:, 0:G, 1:7], a_sb[:, 0:G, 0:6])
            nc.vector.tensor_copy(e2[:, 0:G, 0:1], a_sb[:, 0:G, 2:3])
            nc.vector.tensor_copy(e2[:, 0:G, 7:8], a_sb[:, 0:G, 5:6])
            nc.vector.tensor_tensor(den[:, 0:G], a_sb[:, 0:G], e1[:, 0:G], add)
            nc.vector.tensor_tensor(den[:, 0:G], den[:, 0:G], e2[:, 0:G], add)
            nc.vector.reciprocal_approx_fast(rden[:, 0:G], den[:, 0:G])
            nc.vector.tensor_tensor(b1[:, 0:G], e1[:, 0:G], rden[:, 0:G], mult)
            nc.vector.tensor_tensor(b2[:, 0:G], e2[:, 0:G], rden[:, 0:G], mult)

            # ---- combine: d = b1*z[J1] + b2*z[J2] in ONE custom DVE op per
            # row, then o = d + z as one batched 2x-rate tensor_tensor ----
            t_sb = tpool.tile([P, SUPER, R, D], f16, tag="t")
            o_sb = opool.tile([P, SUPER, R, D], f16, tag="o")
            for g in range(G):
                for r in range(R):
                    nc.vector._custom_dve(
                        wpair,
                        out=t_sb[:, g, r, :],
                        in0=z_sb[:, g, J1[r], :],
                        in1=z_sb[:, g, J2[r], :],
                        s0=b1[:, g, r : r + 1],
                        s1=b2[:, g, r : r + 1],
                    )
            nc.vector.tensor_tensor(
                o_sb[:, 0:G], t_sb[:, 0:G], z_sb[:, 0:G], add
            )
            nc.sync.dma_start(out=o_d[:, s0 : s0 + G], in_=o_sb[:, 0:G])
            s0 += G

    nc.finalize()
    return nc


_NC_CACHE = None


def _get_nc():
    global _NC_CACHE
    if _NC_CACHE is None:
        _NC_CACHE = _build_nc()
    return _NC_CACHE


def _prepare_in_maps(z, W1, W2):
    z = np.asarray(z, dtype=np.float32)
    n = z.shape[0]
    zp = np.zeros((N_PAD, R, D), np.float16)
    zp[:n] = z.astype(np.float16)
    # [core, group, n128, r, d]
    z5 = zp.reshape(N_CORES, GROUPS, P, R, D)
    # node-major natural copy: [core][n128, group, r, d]
    z_nm = np.ascontiguousarray(z5.transpose(0, 2, 1, 3, 4))
    # shipped transposed chunk1: [core][dp, group, r, n128]
    zt1 = np.ascontiguousarray(z5[..., P:D].transpose(0, 4, 1, 3, 2))

    w1t = np.ascontiguousarray(
        np.asarray(W1, np.float32).T.reshape(2, P, 16).transpose(1, 0, 2)
    ).astype(np.float16)  # [128, 2, 16]
    w2 = np.ascontiguousarray(
        np.repeat(np.asarray(W2, np.float32).reshape(16, 1), 2, axis=1)
    ).astype(np.float16)  # [16, 2]
    ident = np.eye(P, dtype=np.float16)

    in_maps = []
    for c in range(N_CORES):
        in_maps.append(
            {
                "z": z_nm[c],
                "zt1": zt1[c],
                "w1t": w1t,
                "w2": w2,
                "ident": ident,
            }
        )
    return in_maps


def _gather_out(res, n):
    # o: [core][n128, group, r, d] -> [N_PAD, R, D]
    out = np.empty((N_CORES, P, GROUPS, R, D), np.float16)
    for c in range(N_CORES):
        out[c] = res.results[c]["o"].reshape(P, GROUPS, R, D)
    full = out.transpose(0, 2, 1, 3, 4).reshape(N_PAD, R, D)
    return full[:n].astype(np.float32)


def kernel(z, W1, W2):
    from concourse.bass_utils import run_bass_kernel_spmd

    nc = _get_nc()
    in_maps = _prepare_in_maps(z, W1, W2)
    res = run_bass_kernel_spmd(nc, in_maps, core_ids=list(range(N_CORES)))
    return _gather_out(res, np.asarray(z).shape[0])


# revision 18
# speedup vs baseline: 1.4686x; 1.0118x over previous
"""Trainium2 Bass kernel for nn_Attentioncross (gnn_message_passing).

Reference computation, per node n (N=50000) and row r (R=8), D=256:
    idx[r] = [r, r+1, r-1] (with idx[0]=[0,1,2], idx[7]=[7,6,5])
    s[n,j]   = W2 @ leaky_relu(W1 @ z[n,j,:], 0.01)        (scalar per row)
    beta     = softmax([s[self], s[j1], s[j2]])            (over the 3)
    o[n,r,:] = z[n,r,:] + beta1*z[n,j1,:] + beta2*z[n,j2,:]

Strategy: data-parallel over N across 8 cores, NODE-MAJOR layout on chip:
SBUF partition = node (128 nodes per group), free = (row, d). All neighbor
references become free-axis offsets — no gather matmuls, no masks. The
combine runs as one custom fused DVE op per row (WPAIR: b1*z[j1]+b2*z[j2]
with per-node beta scalars, registered at import) plus a single batched
2x-rate tensor_tensor residual add per supertile. The score path needs z^T (d on partitions):
d-chunk1 (d=128:256) is shipped pre-transposed fp16 from the host; chunk0
is transposed on-chip by PE matmuls against an identity (lhsT = z slice),
evacuated psum->SBUF by ScalarE. Scores s = W2 @ leaky(W1 @ z^T) come out
of per-row matmuls as [128 nodes, 1]; softmax runs on [128, G, 8] tiles
with shifted free-slices for the neighbor exps. Output is fp16 (rel err
~1e-3 << 2e-2 gate); all matmul accumulation is fp32 in PSUM.
"""
import sys

for p in ("/opt/trn_rl_repo",):
    if p not in sys.path:
        sys.path.insert(0, p)

import numpy as np
from contextlib import ExitStack

N_FULL, R, D = 50000, 8, 256
N_CORES = 8
P = 128
GROUPS = 49                  # node-groups of 128 nodes per core
NODES_PER_CORE = GROUPS * P  # 6272
N_PAD = NODES_PER_CORE * N_CORES   # 50176
SUPER = 5                    # groups per supertile


def _ensure_wpair_op():
    """Register (once) a custom DVE op: out = in0*s0 + in1*s1 with
    per-partition scalars — the full weighted-neighbor sum in one
    instruction. The uop sha is computed with the same lower() the
    compiler uses, so the golden check is self-consistent."""
    from concourse import dve_ops as dops
    from concourse.dve_spec import Spec, Src0, Src1, C0, C1
    from concourse.dve_spec import _has_src1, lower
    from concourse.dve_uop import DveOpSpec

    name = "WPAIR_ANT"
    for o in dops.OPS:
        if o.name == name:
            return o
    spec = Spec(
        body=Src0 * C0 + Src1 * C1,
        reference=lambda in0, in1, s0, s1, imm2: (
            in0.astype(np.float32) * s0 + in1.astype(np.float32) * s1
        ),
    )
    shas = {}
    for ver in ("v3", "v4"):
        tmp = DveOpSpec(
            name=name, opcode=1, uops=lower(spec, ver=ver), rd1_en=_has_src1(spec)
        )
        shas[ver] = tmp.sha(ver)
    op = dops.DveOp(name, spec, subdim=False, uops_sha=shas)
    dops.OPS.append(op)
    dops._SUB_OPCODE_FOR_NAME[name] = dops._CUSTOM_DVE_ROW_BASE + len(dops.OPS) - 1
    return op

# neighbor row indices (matches reference._neighbor_idx for R=8)
J1 = [1, 2, 3, 4, 5, 6, 7, 6]
J2 = [2, 0, 1, 2, 3, 4, 5, 5]


def _build_nc():
    import concourse.bacc as bacc
    import concourse.tile as tile
    from concourse import mybir

    f32 = mybir.dt.float32
    f16 = mybir.dt.float16

    nc = bacc.Bacc("TRN2", target_bir_lowering=False)
    z_d = nc.declare_dram_parameter("z", [P, GROUPS, R, D], f16, isOutput=False)
    zt1_d = nc.declare_dram_parameter("zt1", [P, GROUPS, R, P], f16, isOutput=False)
    w1t_d = nc.declare_dram_parameter("w1t", [P, 2, 16], f16, isOutput=False)
    w2_d = nc.declare_dram_parameter("w2", [16, 2], f16, isOutput=False)
    id_d = nc.declare_dram_parameter("ident", [P, P], f16, isOutput=False)
    o_d = nc.declare_dram_parameter("o", [P, GROUPS, R, D], f16, isOutput=True)

    Prelu = mybir.ActivationFunctionType.Prelu
    Exp = mybir.ActivationFunctionType.Exp
    Copy = mybir.ActivationFunctionType.Copy
    add = mybir.AluOpType.add
    mult = mybir.AluOpType.mult
    wpair = _ensure_wpair_op()

    with tile.TileContext(nc) as tc, ExitStack() as ctx:
        consts = ctx.enter_context(tc.tile_pool(name="consts", bufs=1))
        zpool = ctx.enter_context(tc.tile_pool(name="zp", bufs=3))
        ztpool = ctx.enter_context(tc.tile_pool(name="ztp", bufs=2))
        htpool = ctx.enter_context(tc.tile_pool(name="htp", bufs=2))
        opool = ctx.enter_context(tc.tile_pool(name="op", bufs=2))
        small = ctx.enter_context(tc.tile_pool(name="small", bufs=2))

        ps_zt = ctx.enter_context(tc.tile_pool(name="ps_zt", bufs=3, space="PSUM"))
        ps_ht = ctx.enter_context(tc.tile_pool(name="ps_ht", bufs=2, space="PSUM"))
        ps_sc = ctx.enter_context(tc.tile_pool(name="ps_sc", bufs=2, space="PSUM"))

        id_sb = consts.tile([P, P], f16)
        w1t_sb = consts.tile([P, 2, 16], f16)
        w2_sb = consts.tile([16, 2], f16)
        nc.sync.dma_start(out=id_sb, in_=id_d[:])
        nc.sync.dma_start(out=w1t_sb, in_=w1t_d[:])
        nc.sync.dma_start(out=w2_sb, in_=w2_d[:])
        # warm the exp_and_others activation table set off the critical path
        warm = consts.tile([P, 1], f32)
        nc.scalar.activation(warm, id_sb[:, 0:1], Exp)

        # ramped schedule: small first supers so the first betas land fast
        # (cuts the ~23us DVE prologue stall), small last super for drain
        sched = [1, 2] + [SUPER] * 9 + [1]
        assert sum(sched) == GROUPS
        s0 = 0
        for G in sched:

            z_sb = zpool.tile([P, SUPER, R, D], f16, tag="z")
            nc.sync.dma_start(out=z_sb[:, 0:G], in_=z_d[:, s0 : s0 + G])
            # zt layout: [dp, chunk, group, r, n]; chunk1 shipped from host
            zt_sb = ztpool.tile([P, 2, SUPER, R, P], f16, tag="zt")
            nc.sync.dma_start(
                out=zt_sb[:, 1, 0:G], in_=zt1_d[:, s0 : s0 + G]
            )

            ht_sb = htpool.tile([16, SUPER, R, P], f16, tag="ht")
            sc_ps = ps_sc.tile([P, SUPER, R, 2], f32, tag="sc")

            for g in range(G):
                # ---- transpose chunk0 on PE: zt0[d, n] = z[n, d]^T ----
                for rh in range(2):
                    zt_ps = ps_zt.tile([P, 4, P], f32, tag="ztps")
                    for rr in range(4):
                        r = rh * 4 + rr
                        nc.tensor.matmul(
                            zt_ps[:, rr, :],
                            z_sb[:, g, r, 0:P],
                            id_sb,
                            start=True,
                            stop=True,
                        )
                    nc.scalar.activation(
                        zt_sb[:, 0, g, rh * 4 : rh * 4 + 4, :], zt_ps, Copy
                    )

                # ---- ht = W1 @ z^T (accumulate over the 2 d-chunks) ----
                for rh in range(2):
                    ht_ps = ps_ht.tile([16, 4, P], f32, tag="htps")
                    for c in range(2):
                        nc.tensor.matmul(
                            ht_ps,
                            w1t_sb[:, c, :],
                            zt_sb[:, c, g, rh * 4 : rh * 4 + 4, :],
                            start=(c == 0),
                            stop=(c == 1),
                        )
                    nc.scalar.activation(
                        ht_sb[:, g, rh * 4 : rh * 4 + 4, :],
                        ht_ps,
                        Prelu,
                        alpha=0.01,
                    )

                # ---- scores: s[n] = W2 @ leaky_ht, per row ----
                for r in range(R):
                    nc.tensor.matmul(
                        sc_ps[:, g, r, :],
                        ht_sb[:, g, r, :],
                        w2_sb,
                        start=True,
                        stop=True,
                    )

            # ---- softmax over {self, left(+1 style), right} via row shifts --
            a_sb = small.tile([P, SUPER, R], f32, tag="a")
            e1 = small.tile([P, SUPER, R], f32, tag="e1")
            e2 = small.tile([P, SUPER, R], f32, tag="e2")
            den = small.tile([P, SUPER, R], f32, tag="den")
            rden = small.tile([P, SUPER, R], f32, tag="rden")
            b1 = small.tile([P, SUPER, R], f32, tag="b1")
            b2 = small.tile([P, SUPER, R], f32, tag="b2")

            nc.scalar.activation(a_sb[:, 0:G], sc_ps[:, 0:G, :, 0], Exp)
            # e1[r] = a[J1[r]] : J1 = r+1 for r<7, 6 at r=7
            nc.vector.tensor_copy(e1[:, 0:G, 0:7], a_sb[:, 0:G, 1:8])
            nc.vector.tensor_copy(e1[:, 0:G, 7:8], a_sb[:, 0:G, 6:7])
            # e2[r] = a[J2[r]] : J2 = r-1 for 1<=r<=6, 2 at r=0, 5 at r=7
            nc.vector.tensor_copy(e2[:, 0:G, 1:7], a_sb[:, 0:G, 0:6])
            nc.vector.tensor_copy(e2[:, 0:G, 0:1], a_sb[:, 0:G, 2:3])
            nc.vector.tensor_copy(e2[:, 0:G, 7:8], a_sb[:, 0:G, 5:6])
            nc.vector.tensor_tensor(den[:, 0:G], a_sb[:, 0:G], e1[:, 0:G], add)
            nc.vector.tensor_tensor(den[:, 0:G], den[:, 0:G], e2[:, 0:G], add)
            nc.vector.reciprocal_approx_fast(rden[:, 0:G], den[:, 0:G])
            nc.vector.tensor_tensor(b1[:, 0:G], e1[:, 0:G], rden[:, 0:G], mult)
            nc.vector.tensor_tensor(b2[:, 0:G], e2[:, 0:G], rden[:, 0:G], mult)

            # ---- combine: d = b1*z[J1] + b2*z[J2] in ONE custom DVE op per
            # row, then o = d + z as one batched 2x-rate tensor_tensor ----
            o_sb = opool.tile([P, SUPER, R, D], f16, tag="o")
            for g in range(G):
                for r in range(R):
                    nc.vector._custom_dve(
                        wpair,
                        out=o_sb[:, g, r, :],
                        in0=z_sb[:, g, J1[r], :],
                        in1=z_sb[:, g, J2[r], :],
                        s0=b1[:, g, r : r + 1],
                        s1=b2[:, g, r : r + 1],
                    )
            nc.vector.tensor_tensor(
                o_sb[:, 0:G], o_sb[:, 0:G], z_sb[:, 0:G], add
            )
            nc.sync.dma_start(out=o_d[:, s0 : s0 + G], in_=o_sb[:, 0:G])
            s0 += G

    nc.finalize()
    return nc


_NC_CACHE = None


def _get_nc():
    global _NC_CACHE
    if _NC_CACHE is None:
        _NC_CACHE = _build_nc()
    return _NC_CACHE


def _prepare_in_maps(z, W1, W2):
    z = np.asarray(z, dtype=np.float32)
    n = z.shape[0]
    zp = np.zeros((N_PAD, R, D), np.float16)
    zp[:n] = z.astype(np.float16)
    # [core, group, n128, r, d]
    z5 = zp.reshape(N_CORES, GROUPS, P, R, D)
    # node-major natural copy: [core][n128, group, r, d]
    z_nm = np.ascontiguousarray(z5.transpose(0, 2, 1, 3, 4))
    # shipped transposed chunk1: [core][dp, group, r, n128]
    zt1 = np.ascontiguousarray(z5[..., P:D].transpose(0, 4, 1, 3, 2))

    w1t = np.ascontiguousarray(
        np.asarray(W1, np.float32).T.reshape(2, P, 16).transpose(1, 0, 2)
    ).astype(np.float16)  # [128, 2, 16]
    w2 = np.ascontiguousarray(
        np.repeat(np.asarray(W2, np.float32).reshape(16, 1), 2, axis=1)
    ).astype(np.float16)  # [16, 2]
    ident = np.eye(P, dtype=np.float16)

    in_maps = []
    for c in range(N_CORES):
        in_maps.append(
            {
                "z": z_nm[c],
                "zt1": zt1[c],
                "w1t": w1t,
                "w2": w2,
                "ident": ident,
            }
        )
    return in_maps


def _gather_out(res, n):
    # o: [core][n128, group, r, d] -> [N_PAD, R, D]
    out = np.empty((N_CORES, P, GROUPS, R, D), np.float16)
    for c in range(N_CORES):
        out[c] = res.results[c]["o"].reshape(P, GROUPS, R, D)
    full = out.transpose(0, 2, 1, 3, 4).reshape(N_PAD, R, D)
    return full[:n].astype(np.float32)


def kernel(z, W1, W2):
    from concourse.bass_utils import run_bass_kernel_spmd

    nc = _get_nc()
    in_maps = _prepare_in_maps(z, W1, W2)
    res = run_bass_kernel_spmd(nc, in_maps, core_ids=list(range(N_CORES)))
    return _gather_out(res, np.asarray(z).shape[0])
